# revision 5
# baseline (speedup 1.0000x reference)
"""MoE feed-forward (top-2 of 8 experts) Trainium2 Bass kernel.

Strategy: expert-parallel across 8 NeuronCores. Each core holds one
expert's W1/b1/W2/b2, x is replicated. Every core computes the gating
(fp32, exact top-2 selection), runs its expert densely over all tokens
with the combine weight applied (zero for unrouted tokens), then a
ReduceScatter sums the per-expert partials and leaves each core with a
512-token shard, on which it applies residual + LayerNorm. The host
concatenates the 8 shards.

FFN matmuls run in bf16 (fp32 accumulate); routing runs in fp32 so the
top-2 selection matches the fp32 reference exactly.
"""

from contextlib import ExitStack

import numpy as np
import ml_dtypes

import concourse.bass as bass
import concourse.bacc as bacc
import concourse.tile as tile
from concourse import mybir
from concourse.bass_utils import run_bass_kernel_spmd

FP32 = mybir.dt.float32
BF16 = mybir.dt.bfloat16
AF = mybir.ActivationFunctionType
ALU = mybir.AluOpType

B, T, D, H, E = 2, 2048, 1024, 4096, 8
N = B * T            # 4096 tokens
NCORES = 8
TPC = N // NCORES    # 512 tokens output shard per core
P = 128
KD = D // P          # 8 contraction tiles over D
KH = H // P          # 32 contraction tiles over H
G = 256              # FFN token group
NG = N // G          # 16 groups
NT = N // P          # 32 token tiles (routing)
LN_EPS = 1e-5


def build_program():
    nc = bacc.Bacc("TRN2", target_bir_lowering=False, num_devices=NCORES)

    xT = nc.dram_tensor("xT", [D, N], FP32, kind="ExternalInput")
    xTb = nc.dram_tensor("xTb", [D, N], BF16, kind="ExternalInput")
    xs = nc.dram_tensor("xs", [TPC, D], FP32, kind="ExternalInput")
    Wg = nc.dram_tensor("Wg", [D, E], FP32, kind="ExternalInput")
    bg = nc.dram_tensor("bg", [1, E], FP32, kind="ExternalInput")
    W1 = nc.dram_tensor("W1e", [D, H], BF16, kind="ExternalInput")
    b1 = nc.dram_tensor("b1e", [1, H], FP32, kind="ExternalInput")
    W2 = nc.dram_tensor("W2e", [H, D], BF16, kind="ExternalInput")
    b2 = nc.dram_tensor("b2e", [1, D], FP32, kind="ExternalInput")
    eoh = nc.dram_tensor("eoh", [1, E], FP32, kind="ExternalInput")
    gam = nc.dram_tensor("gamma", [1, D], FP32, kind="ExternalInput")
    bet = nc.dram_tensor("beta", [1, D], FP32, kind="ExternalInput")
    out = nc.dram_tensor("out", [TPC, D], FP32, kind="ExternalOutput")

    xT_t = xT.rearrange("(kd p) n -> p kd n", p=P)      # [128, KD, N]
    xTb_t = xTb.rearrange("(kd p) n -> p kd n", p=P)    # [128, KD, N]
    Wg_t = Wg.rearrange("(kd p) e -> p kd e", p=P)      # [128, KD, E]
    W1_t = W1.rearrange("(kd p) h -> p kd h", p=P)      # [128, KD, H]
    W2_t = W2.rearrange("(hk p) d -> p hk d", p=P)      # [128, KH, D]
    b1_t = b1.rearrange("o (hk p) -> p (o hk)", p=P)    # [128, KH]

    with ExitStack() as ctx:
        tc = ctx.enter_context(tile.TileContext(nc))
        singles = ctx.enter_context(tc.tile_pool(name="singles", bufs=1))
        xf_pool = ctx.enter_context(tc.tile_pool(name="xf", bufs=2))
        xb_pool = ctx.enter_context(tc.tile_pool(name="xb", bufs=2))
        rt_pool = ctx.enter_context(tc.tile_pool(name="rt", bufs=4))
        h_pool = ctx.enter_context(tc.tile_pool(name="h", bufs=1))
        y_pool = ctx.enter_context(tc.tile_pool(name="y", bufs=2))
        ps_small = ctx.enter_context(
            tc.tile_pool(name="ps_small", bufs=2, space="PSUM"))
        ps_h = ctx.enter_context(tc.tile_pool(name="ps_h", bufs=2, space="PSUM"))
        ps_y = ctx.enter_context(tc.tile_pool(name="ps_y", bufs=2, space="PSUM"))
        dram = ctx.enter_context(tc.tile_pool(name="dram", bufs=1, space="DRAM"))
        ln_pool = ctx.enter_context(tc.tile_pool(name="ln", bufs=2))

        # ---- resident constants -------------------------------------------
        W1sb = singles.tile([P, KD, H], BF16)
        nc.sync.dma_start(out=W1sb[:], in_=W1_t[:])
        W2sb = singles.tile([P, KH, D], BF16)
        nc.sync.dma_start(out=W2sb[:], in_=W2_t[:])
        Wgsb = singles.tile([P, KD, E], FP32)
        nc.sync.dma_start(out=Wgsb[:], in_=Wg_t[:])
        b1sb = singles.tile([P, KH], FP32)
        nc.sync.dma_start(out=b1sb[:], in_=b1_t[:])
        b2sb = singles.tile([P, D], FP32)
        nc.sync.dma_start(out=b2sb[:], in_=b2[:].to_broadcast([P, D]))
        bgsb = singles.tile([P, E], FP32)
        nc.sync.dma_start(out=bgsb[:], in_=bg[:].to_broadcast([P, E]))
        eohsb = singles.tile([P, E], FP32)
        nc.sync.dma_start(out=eohsb[:], in_=eoh[:].to_broadcast([P, E]))
        gamsb = singles.tile([P, D], FP32)
        nc.sync.dma_start(out=gamsb[:], in_=gam[:].to_broadcast([P, D]))
        betsb = singles.tile([P, D], FP32)
        nc.sync.dma_start(out=betsb[:], in_=bet[:].to_broadcast([P, D]))
        epssb = singles.tile([P, 1], FP32)
        nc.vector.memset(epssb[:], LN_EPS)
        wall = singles.tile([P, NT], FP32)  # combine weight per token tile

        partial = dram.tile([N, D], FP32)
        rs_out = dram.tile([TPC, D], FP32)

        # ---- phase 1: routing (fp32) --------------------------------------
        for ti in range(NT):
            xf = xf_pool.tile([P, KD, P], FP32)
            nc.sync.dma_start(out=xf[:], in_=xT_t[:, :, ti * P:(ti + 1) * P])
            lg_ps = ps_small.tile([P, E], FP32, space="PSUM")
            for kd in range(KD):
                nc.tensor.matmul(
                    out=lg_ps[:],
                    lhsT=xf[:, kd, :],
                    rhs=Wgsb[:, kd, :],
                    start=(kd == 0),
                    stop=(kd == KD - 1),
                )
            logits = rt_pool.tile([P, E], FP32, tag="logits")
            nc.vector.tensor_add(out=logits[:], in0=lg_ps[:], in1=bgsb[:])

            m1 = rt_pool.tile([P, 1], FP32, tag="m1")
            nc.vector.reduce_max(out=m1[:], in_=logits[:],
                                 axis=mybir.AxisListType.X)
            mask1 = rt_pool.tile([P, E], FP32, tag="mask1")
            nc.vector.tensor_scalar(
                out=mask1[:], in0=logits[:], scalar1=m1[:], scalar2=None,
                op0=ALU.is_equal)
            # knock out the argmax and find the runner-up
            neg = rt_pool.tile([P, E], FP32, tag="neg")
            nc.scalar.mul(neg[:], mask1[:], -1e30)
            lm = rt_pool.tile([P, E], FP32, tag="lm")
            nc.vector.tensor_add(out=lm[:], in0=logits[:], in1=neg[:])
            m2 = rt_pool.tile([P, 1], FP32, tag="m2")
            nc.vector.reduce_max(out=m2[:], in_=lm[:],
                                 axis=mybir.AxisListType.X)
            mask2 = rt_pool.tile([P, E], FP32, tag="mask2")
            nc.vector.tensor_scalar(
                out=mask2[:], in0=lm[:], scalar1=m2[:], scalar2=None,
                op0=ALU.is_equal)
            # softmax over the two selected logits:
            # s1 = 1/(1+exp(m2-m1)), s2 = exp(m2-m1) * s1
            dlt = rt_pool.tile([P, 1], FP32, tag="dlt")
            nc.vector.tensor_tensor(out=dlt[:], in0=m2[:], in1=m1[:],
                                    op=ALU.subtract)
            ex = rt_pool.tile([P, 1], FP32, tag="ex")
            nc.scalar.activation(out=ex[:], in_=dlt[:], func=AF.Exp)
            s1 = rt_pool.tile([P, 1], FP32, tag="s1")
            nc.scalar.add(s1[:], ex[:], 1.0)
            nc.vector.reciprocal(out=s1[:], in_=s1[:])
            s2 = rt_pool.tile([P, 1], FP32, tag="s2")
            nc.vector.tensor_tensor(out=s2[:], in0=ex[:], in1=s1[:],
                                    op=ALU.mult)
            # combine weights for all experts, then select this core's expert
            wc1 = rt_pool.tile([P, E], FP32, tag="wc1")
            nc.vector.tensor_scalar_mul(out=wc1[:], in0=mask1[:], scalar1=s1[:])
            wc2 = rt_pool.tile([P, E], FP32, tag="wc2")
            nc.vector.tensor_scalar_mul(out=wc2[:], in0=mask2[:], scalar1=s2[:])
            wc = rt_pool.tile([P, E], FP32, tag="wc")
            nc.vector.tensor_add(out=wc[:], in0=wc1[:], in1=wc2[:])
            nc.vector.tensor_tensor(out=wc[:], in0=wc[:], in1=eohsb[:],
                                    op=ALU.mult)
            nc.vector.reduce_sum(out=wall[:, ti:ti + 1], in_=wc[:],
                                 axis=mybir.AxisListType.X)

        # ---- phase 2: expert FFN (bf16), dense over all tokens ------------
        for g in range(NG):
            xb = xb_pool.tile([P, KD, G], BF16)
            nc.sync.dma_start(out=xb[:], in_=xTb_t[:, :, g * G:(g + 1) * G])
            hT = h_pool.tile([P, KH, G], BF16)
            for hk in range(KH):
                h_ps = ps_h.tile([P, G], FP32, space="PSUM")
                for kd in range(KD):
                    nc.tensor.matmul(
                        out=h_ps[:],
                        lhsT=W1sb[:, kd, hk * P:(hk + 1) * P],
                        rhs=xb[:, kd, :],
                        start=(kd == 0),
                        stop=(kd == KD - 1),
                    )
                # gelu(h + b1) straight out of PSUM, cast to bf16
                nc.scalar.activation(
                    out=hT[:, hk, :], in_=h_ps[:], func=AF.Gelu,
                    bias=b1sb[:, hk:hk + 1], scale=1.0)
            for tsub in range(G // P):
                ti = g * (G // P) + tsub
                y_ps = ps_y.tile([P, D], FP32, space="PSUM")
                for hk in range(KH):
                    lhsT = hT[:, hk, tsub * P:(tsub + 1) * P]
                    for dh in range(2):
                        nc.tensor.matmul(
                            out=y_ps[:, dh * 512:(dh + 1) * 512],
                            lhsT=lhsT,
                            rhs=W2sb[:, hk, dh * 512:(dh + 1) * 512],
                            start=(hk == 0),
                            stop=(hk == KH - 1),
                        )
                y_sb = y_pool.tile([P, D], FP32)
                nc.vector.tensor_add(out=y_sb[:], in0=y_ps[:], in1=b2sb[:])
                nc.vector.tensor_scalar_mul(
                    out=y_sb[:], in0=y_sb[:], scalar1=wall[:, ti:ti + 1])
                nc.sync.dma_start(out=partial[ti * P:(ti + 1) * P, :],
                                  in_=y_sb[:])

        # ---- phase 3: sum expert partials, keep this core's token shard ---
        nc.gpsimd.collective_compute(
            "ReduceScatter",
            ALU.add,
            replica_groups=[list(range(NCORES))],
            ins=[partial.opt()],
            outs=[rs_out.opt()],
        )

        # ---- phase 4: residual + LayerNorm on the 512-token shard ---------
        for ti in range(TPC // P):
            r = ln_pool.tile([P, D], FP32, tag="r")
            nc.sync.dma_start(out=r[:], in_=rs_out[ti * P:(ti + 1) * P, :])
            xr = ln_pool.tile([P, D], FP32, tag="xr")
            nc.sync.dma_start(out=xr[:], in_=xs[ti * P:(ti + 1) * P, :])
            nc.vector.tensor_add(out=r[:], in0=r[:], in1=xr[:])

            stats = ln_pool.tile([P, 2, 6], FP32, tag="stats")
            rr = r[:].rearrange("p (s f) -> p s f", s=2)
            for s in range(2):
                nc.vector.bn_stats(out=stats[:, s, :], in_=rr[:, s, :])
            mv = ln_pool.tile([P, 2], FP32, tag="mv")
            nc.vector.bn_aggr(out=mv[:], in_=stats[:])
            rstd = ln_pool.tile([P, 1], FP32, tag="rstd")
            nc.scalar.activation(out=rstd[:], in_=mv[:, 1:2], func=AF.Sqrt,
                                 bias=epssb[:], scale=1.0)
            nc.vector.reciprocal(out=rstd[:], in_=rstd[:])
            nc.vector.tensor_scalar(
                out=r[:], in0=r[:], scalar1=mv[:, 0:1], scalar2=rstd[:],
                op0=ALU.subtract, op1=ALU.mult)
            nc.vector.tensor_tensor(out=r[:], in0=r[:], in1=gamsb[:],
                                    op=ALU.mult)
            nc.vector.tensor_add(out=r[:], in0=r[:], in1=betsb[:])
            nc.sync.dma_start(out=out[ti * P:(ti + 1) * P, :], in_=r[:])

    nc.compile()
    return nc


_NC_CACHE = None


def _get_program():
    global _NC_CACHE
    if _NC_CACHE is None:
        _NC_CACHE = build_program()
    return _NC_CACHE


def make_in_maps(x, Wg, bg, W1, b1, W2, b2, gamma, beta):
    xf = np.ascontiguousarray(x.reshape(N, D).astype(np.float32))
    xT = np.ascontiguousarray(xf.T)                      # [D, N]
    xTb = xT.astype(ml_dtypes.bfloat16)
    Wg2 = np.ascontiguousarray(Wg.astype(np.float32))
    bg2 = np.ascontiguousarray(bg.astype(np.float32).reshape(1, E))
    gam = np.ascontiguousarray(gamma.astype(np.float32).reshape(1, D))
    bet = np.ascontiguousarray(beta.astype(np.float32).reshape(1, D))
    in_maps = []
    for e in range(NCORES):
        onehot = np.zeros((1, E), np.float32)
        onehot[0, e] = 1.0
        in_maps.append({
            "xT": xT,
            "xTb": xTb,
            "xs": np.ascontiguousarray(xf[e * TPC:(e + 1) * TPC]),
            "Wg": Wg2,
            "bg": bg2,
            "W1e": np.ascontiguousarray(W1[e].astype(ml_dtypes.bfloat16)),
            "b1e": np.ascontiguousarray(b1[e].astype(np.float32).reshape(1, H)),
            "W2e": np.ascontiguousarray(W2[e].astype(ml_dtypes.bfloat16)),
            "b2e": np.ascontiguousarray(b2[e].astype(np.float32).reshape(1, D)),
            "eoh": onehot,
            "gamma": gam,
            "beta": bet,
        })
    return in_maps


def kernel(x, Wg, bg, W1, b1, W2, b2, gamma, beta, _trace=False):
    nc = _get_program()
    in_maps = make_in_maps(x, Wg, bg, W1, b1, W2, b2, gamma, beta)
    res = run_bass_kernel_spmd(
        nc, in_maps, core_ids=list(range(NCORES)), trace=_trace)
    outs = [res.results[c]["out"] for c in range(NCORES)]
    full = np.concatenate(outs, axis=0).reshape(B, T, D).astype(np.float32)
    if _trace:
        kernel.last_results = res
    return full
